# revision 10
# baseline (speedup 1.0000x reference)
"""Trainium2 Bass kernel for nn_GroupLinear: channel-shuffled grouped MLP.

Computes, for x [4096, 16384]:
    h = relu(einsum('bgi,gio->bgo', x[:, perm1].reshape(B,128,128), W1) + b1)
    h = relu(einsum('bgi,gio->bgo', h.reshape(B,8192)[:, perm2].reshape(B,128,64), W2) + b2)
    y = sigmoid(((h.reshape(B,4096) @ W3 + b3) @ W4 + b4) @ W5 + b5)

Sharding: data-parallel over batch across 8 cores (512 rows/core); weights
replicated. The dense head has no intermediate nonlinearity, so W3@W4@W5 is
collapsed on the host into a single [4096,1] vector (exact algebra).

Per-core pipeline:
  A) transpose x shard via PE into DRAM scratch xT [16384, nb] bf16,
     processed in batch halves; 4 transposed chunks packed per DMA write
     (3D-AP) on the ACT HWDGE ring.
  B) GL1: batched dma_gather (2048 perm1 rows / call, full-batch 1KiB
     rows) -> per-group matmuls vs W1 (pairs packed per PSUM bank) ->
     relu+bias -> h1 [8192, nb] bf16 in DRAM (coalesced writes, SP ring)
  C) GL2: dma_gather of perm2 rows -> block-diag W2 pair matmuls ->
     quad-packed h2 tiles resident in SBUF (bf16)
  D) head: 32 accumulating matmuls vs collapsed W3@W4@W5 -> sigmoid -> y

Phases B-D operate on the full per-core batch (fat 1KiB gather rows,
half the instruction count); phase A is split into batch halves and
pipelines against the previous rep's B-D via rotating DRAM staging
buffers (bufs=2). Gathers ride the GPSIMD SWDGE queue (one descriptor
per 16 rows), x loads the SP HWDGE ring, xT writes the ACT ring.
"""

import hashlib

import numpy as np

import concourse.bass as bass
import concourse.mybir as mybir
import concourse.tile as tile
from concourse import bacc, bass_utils, library_config
from concourse.masks import make_identity

G = 128          # groups
C1 = G * 128     # 16384 input channels
C2 = G * 64      # 8192 channels after GL1
N_CORES = 8
B_FULL = 4096

F32 = mybir.dt.float32
BF16 = mybir.dt.bfloat16
I16 = mybir.dt.int16

K1 = 2048        # perm1 idxs per dma_gather call
K2 = 2048        # perm2 idxs per dma_gather call


def build_nc(nb: int, b345: float, reps: int = 1, mid_bf16: int = 1,
             cblk: int = 1024, ablate: int = 0):
    """Build the per-core Bass program for batch-per-core nb.

    reps > 1 repeats the whole pipeline inside one NEFF (for timing:
    device time per rep = (T_reps - T_1) / (reps - 1), cancelling the
    fixed per-execution dispatch overhead).
    """
    nh = 2 if nb >= 512 else 1    # batch halves for phase A
    nbh = nb // nh
    assert nbh % 128 == 0
    nbt = nbh // 128              # batch subtiles of 128 per half
    ncb = C1 // cblk              # channel blocks in phase A
    ncc = cblk // 128             # 128-channel chunks per block

    nc = bacc.Bacc("TRN2", debug=False)
    xs = nc.dram_tensor("xs", [nb, C1], F32, kind="ExternalInput")
    w1 = nc.dram_tensor("w1", [128, G * 64], BF16, kind="ExternalInput")
    w2 = nc.dram_tensor("w2", [128, 64 * 64], BF16, kind="ExternalInput")
    w345 = nc.dram_tensor("w345", [128, 32], BF16, kind="ExternalInput")
    b1p = nc.dram_tensor("b1p", [128, 64], F32, kind="ExternalInput")
    b2q = nc.dram_tensor("b2q", [128, 32], F32, kind="ExternalInput")
    p1g = nc.dram_tensor("p1g", [128, C1 // 16], I16, kind="ExternalInput")
    p2g = nc.dram_tensor("p2g", [128, C2 // 16], I16, kind="ExternalInput")
    y = nc.dram_tensor("y", [1, nb], F32, kind="ExternalOutput")

    relu_t = mybir.ActivationFunctionType.Relu
    sigm_t = mybir.ActivationFunctionType.Sigmoid

    with tile.TileContext(nc) as tc:
        with (
            tc.tile_pool(name="const", bufs=1) as cpool,
            tc.tile_pool(name="h2p", bufs=1) as h2pool,
            tc.tile_pool(name="work", bufs=2) as pool,
            tc.tile_pool(name="psum", bufs=2, space="PSUM") as psum,
            tc.tile_pool(name="dram", bufs=1, space="DRAM") as dpool,
        ):
            # ---- constants / weights preload ----
            w1s = cpool.tile([128, G * 64], BF16)
            nc.sync.dma_start(w1s[:], w1.ap())
            w2s = cpool.tile([128, 64 * 64], BF16)
            nc.sync.dma_start(w2s[:], w2.ap())
            w345s = cpool.tile([128, 32], BF16)
            nc.sync.dma_start(w345s[:], w345.ap())
            b1s = cpool.tile([128, 64], F32)
            nc.sync.dma_start(b1s[:], b1p.ap())
            b2s = cpool.tile([128, 32], F32)
            nc.sync.dma_start(b2s[:], b2q.ap())
            p1s = cpool.tile([128, C1 // 16], I16)
            nc.sync.dma_start(p1s[:], p1g.ap())
            p2s = cpool.tile([128, C2 // 16], I16)
            nc.sync.dma_start(p2s[:], p2g.ap())
            ident = cpool.tile([128, 128], F32)
            make_identity(nc, ident[:])
            ident16 = cpool.tile([128, 128], BF16)
            make_identity(nc, ident16[:])

            nc.gpsimd.load_library(library_config.mlp)

            for _rep in range(reps):
                run_rep(nc, tc, pool, psum, h2pool, dpool, xs, y,
                        w1s, w2s, w345s, b1s, b2s, p1s, p2s, ident, ident16,
                        nb, nh, nbh, nbt, ncb, ncc, cblk, b345,
                        relu_t, sigm_t, ablate)

    nc.compile()
    return nc


def run_rep(nc, tc, pool, psum, h2pool, dpool, xs, y,
            w1s, w2s, w345s, b1s, b2s, p1s, p2s, ident, ident16,
            nb, nh, nbh, nbt, ncb, ncc, cblk, b345, relu_t, sigm_t,
            ablate=0):
    xTd = dpool.tile([C1, nb], BF16, tag="xTd", name="xTd", bufs=2)
    # h1 stays fully SBUF-resident: one wide tile, channel ch at partition
    # ch%128, cols [(ch//128)*nb, (ch//128+1)*nb) -- the rank/token layout
    # the SBUF-source dma_gather addresses directly.
    h1sb = pool.tile([128, (C2 // 128) * nb], BF16, tag="h1sb", bufs=1,
                     name="h1sb")

    # ---- phase A: transpose x -> xT (DRAM, bf16), in batch halves ----
    # 4 transposed 128-channel chunks are packed side-by-side into one wide
    # SBUF tile and written with a single 3D-AP DMA (ACT ring), cutting the
    # HWDGE instruction count 4x vs per-chunk writes.
    for h in range(0 if ablate == 2 else nh):
        b0 = h * nbh
        for cb in range(ncb):
            xn = []
            for bt in range(nbt):
                t = pool.tile([128, cblk], F32, tag=f"xn{bt}", name=f"xn{bt}")
                nc.sync.dma_start(
                    t[:], xs.ap()[b0 + bt * 128:b0 + (bt + 1) * 128,
                                  cb * cblk:(cb + 1) * cblk])
                xn.append(t)
            for cw in range(ncc // 4):
                wide = pool.tile([128, 4 * nbh], BF16, tag="xtw", bufs=3,
                                 name="xtw")
                for cp in range(2):
                    ps_a = psum.tile([128, 2 * nbh], F32, tag="ps_a")
                    for ci in range(2):
                        cc = cw * 4 + cp * 2 + ci
                        for bt in range(nbt):
                            nc.tensor.transpose(
                                ps_a[:, ci * nbh + bt * 128:
                                     ci * nbh + (bt + 1) * 128],
                                xn[bt][:, cc * 128:(cc + 1) * 128],
                                ident[:])
                    nc.vector.tensor_copy(
                        wide[:, cp * 2 * nbh:(cp + 1) * 2 * nbh], ps_a[:])
                ch0 = cb * cblk + cw * 512
                nc.scalar.dma_start(
                    xTd[ch0:ch0 + 512, b0:b0 + nbh].rearrange(
                        "(c p) b -> p c b", c=4), wide[:])

    if ablate == 1:
        yt0 = pool.tile([1, nb], F32, tag="yt", bufs=2, name="yt0")
        nc.vector.memset(yt0[:], 0.0)
        nc.sync.dma_start(y.ap(), yt0[:])
        return

    # ---- phase B: GL1 (batched gather + paired matmuls), full batch ----
    for k in range(C1 // K1):
        g1 = pool.tile([128, K1 // 128, nb], BF16, tag="g1", name="g1")
        nc.gpsimd.dma_gather(
            g1[:], xTd[:], p1s[:, k * (K1 // 16):(k + 1) * (K1 // 16)],
            K1, K1, nb, single_packet=False)
        for j in range(K1 // 256):          # pairs of groups
            pair = (K1 // 256) * k + j
            ps_b = psum.tile([128, nb], F32, tag="ps_b")
            for half in range(2):
                blk = 2 * j + half
                g = (K1 // 128) * k + blk
                nc.tensor.matmul(
                    ps_b[64 * half:64 * (half + 1), :],
                    lhsT=w1s[:, g * 64:(g + 1) * 64], rhs=g1[:, blk, :],
                    start=True, stop=True,
                    tile_position=(0, 64 * half) if half else None)
            nc.scalar.activation(h1sb[:, pair * nb:(pair + 1) * nb],
                                 ps_b[:], relu_t,
                                 bias=b1s[:, pair:pair + 1])

    # ---- phase C: GL2, full batch. perm2 rows come straight out of SBUF
    # via the transpose-mode gather (batch-major output), then PE flips
    # each [128b x 128ch] block back to channel-major for the matmuls.
    h2 = []
    nbc = nb // 128                         # batch 128-blocks
    for k in range(C2 // K2):
        g2t = pool.tile([128, nbc, K2], BF16, tag="g2t", bufs=1, name="g2t")
        nc.gpsimd.dma_gather(
            g2t[:], h1sb[:], p2s[:, k * (K2 // 16):(k + 1) * (K2 // 16)],
            K2, K2, nb, transpose=True, single_packet=False,
            sbuf_tokens_per_rank=128,
            sbuf_free_dim_per_rank=nb * 2,
            sbuf_free_dim_pad_per_rank=0,
            sbuf_byte_offset=0)
        for j in range(K2 // 256):          # quads of groups
            t = (K2 // 256) * k + j
            rhs2 = []
            for half in range(2):
                blk = 2 * j + half
                ps_r = psum.tile([128, nb], BF16, tag="ps_r")
                for c in range(nbc):
                    nc.tensor.transpose(
                        ps_r[:, c * 128:(c + 1) * 128],
                        g2t[:, c, blk * 128:(blk + 1) * 128],
                        ident16[:])
                r2 = pool.tile([128, nb], BF16, tag="rhs2", bufs=4,
                               name="r2")
                nc.vector.tensor_copy(r2[:], ps_r[:])
                rhs2.append(r2)
            ps_c = psum.tile([128, nb], F32, tag="ps_c", bufs=1)
            for half in range(2):
                q = (K2 // 128) * k + 2 * j + half
                nc.tensor.matmul(
                    ps_c[64 * half:64 * (half + 1), :],
                    lhsT=w2s[:, q * 64:(q + 1) * 64], rhs=rhs2[half][:],
                    start=True, stop=True,
                    tile_position=(0, 64 * half) if half else None)
            h2t = h2pool.tile([128, nb], BF16, tag=f"h2_{t}", name=f"h2_{t}")
            nc.scalar.activation(h2t[:], ps_c[:], relu_t,
                                 bias=b2s[:, t:t + 1])
            h2.append(h2t)

    # ---- phase D: head ----
    ps_d = psum.tile([1, nb], F32, tag="ps_d", bufs=1)
    for t in range(32):
        nc.tensor.matmul(ps_d[:], lhsT=w345s[:, t:t + 1], rhs=h2[t][:],
                         start=(t == 0), stop=(t == 31))
    yt = pool.tile([1, nb], F32, tag="yt", bufs=2)
    nc.scalar.activation(yt[:], ps_d[:], sigm_t, bias=float(b345))
    nc.sync.dma_start(y.ap(), yt[:])


def _gather_table(perm: np.ndarray, chunk: int) -> np.ndarray:
    """Index table for dma_gather: chunk c occupies cols [c*chunk/16,
    (c+1)*chunk/16); within a chunk, idx position i = col*16 + row.
    Rows 0-15 hold the indices; replicated to all 128 partitions."""
    n = perm.shape[0]
    cols = chunk // 16
    t = np.zeros((128, (n // chunk) * cols), dtype=np.int16)
    for c in range(n // chunk):
        blk = perm[c * chunk:(c + 1) * chunk].reshape(cols, 16).T  # [16, cols]
        t[:, c * cols:(c + 1) * cols] = np.tile(blk, (8, 1))
    return t


def prep_host(perm1, perm2, W1, b1, W2, b2, W3, b3, W4, b4, W5, b5,
              mid_bf16=1):
    """Host-side layout prep of weights / index tables (replicated per core)."""
    import ml_dtypes
    wdt = ml_dtypes.bfloat16
    w1h = np.ascontiguousarray(
        W1.astype(np.float32).transpose(1, 0, 2).reshape(128, G * 64)).astype(wdt)
    w2h = np.zeros((128, 64 * 64), dtype=wdt)
    for q in range(64):
        w2h[0:64, q * 64:q * 64 + 32] = W2[2 * q].astype(wdt)
        w2h[64:128, q * 64 + 32:(q + 1) * 64] = W2[2 * q + 1].astype(wdt)
    wv = (W3.astype(np.float64) @ W4.astype(np.float64) @ W5.astype(np.float64))
    w345h = np.ascontiguousarray(
        wv.astype(np.float32).reshape(32, 128).T).astype(wdt)
    b345 = float(
        (b3.astype(np.float64) @ W4.astype(np.float64) @ W5.astype(np.float64)
         + b4.astype(np.float64) @ W5.astype(np.float64)
         + b5.astype(np.float64)).reshape(()))
    b1h = np.ascontiguousarray(
        b1.astype(np.float32).reshape(64, 128).T)   # col k = [b1[2k]; b1[2k+1]]
    b2h = np.ascontiguousarray(
        b2.astype(np.float32).reshape(32, 128).T)   # col j = b2[4j:4j+4] stacked
    p1h = _gather_table(perm1.astype(np.int64), K1)
    p2h = _gather_table(perm2.astype(np.int64), K2)
    return {"w1": w1h, "w2": w2h, "w345": w345h, "b1p": b1h, "b2q": b2h,
            "p1g": p1h, "p2g": p2h}, b345


_NC_CACHE: dict = {}

MID_BF16 = 1


def get_nc(nb: int, b345: float, key_bytes: bytes, mid_bf16: int = MID_BF16):
    key = (nb, mid_bf16,
           hashlib.sha256(key_bytes + np.float64(b345).tobytes()).hexdigest())
    if key not in _NC_CACHE:
        _NC_CACHE[key] = build_nc(nb, b345, mid_bf16=mid_bf16)
    return _NC_CACHE[key]


def kernel(x, perm1, perm2, W1, b1, W2, b2, W3, b3, W4, b4, W5, b5):
    x = np.asarray(x)
    consts, b345 = prep_host(np.asarray(perm1), np.asarray(perm2),
                             np.asarray(W1), np.asarray(b1), np.asarray(W2),
                             np.asarray(b2), np.asarray(W3), np.asarray(b3),
                             np.asarray(W4), np.asarray(b4), np.asarray(W5),
                             np.asarray(b5), mid_bf16=MID_BF16)
    nb = x.shape[0] // N_CORES
    key_bytes = np.asarray(perm1).tobytes() + np.asarray(perm2).tobytes()
    nc = get_nc(nb, b345, key_bytes)
    in_maps = []
    for c in range(N_CORES):
        m = dict(consts)
        m["xs"] = np.ascontiguousarray(x[c * nb:(c + 1) * nb].astype(np.float32))
        in_maps.append(m)
    res = bass_utils.run_bass_kernel_spmd(nc, in_maps, core_ids=list(range(N_CORES)))
    out = np.concatenate([res.results[c]["y"].reshape(nb) for c in range(N_CORES)])
    return out.reshape(-1, 1).astype(np.float32)


# revision 13
# speedup vs baseline: 2.0706x; 2.0706x over previous
"""Trainium2 Bass kernel for nn_GroupLinear: channel-shuffled grouped MLP.

Computes, for x [4096, 16384]:
    h = relu(einsum('bgi,gio->bgo', x[:, perm1].reshape(B,128,128), W1) + b1)
    h = relu(einsum('bgi,gio->bgo', h.reshape(B,8192)[:, perm2].reshape(B,128,64), W2) + b2)
    y = sigmoid(((h.reshape(B,4096) @ W3 + b3) @ W4 + b4) @ W5 + b5)

Sharding: data-parallel over batch across 8 cores (512 rows/core); weights
replicated. The dense head has no intermediate nonlinearity, so W3@W4@W5 is
collapsed on the host into a single [4096,1] vector (exact algebra).

Per-core pipeline:
  A) transpose x shard via PE into DRAM scratch xT [16384, nb] bf16,
     processed in batch halves; 4 transposed chunks packed per DMA write
     (3D-AP) on the ACT HWDGE ring.
  B) GL1: batched dma_gather (2048 perm1 rows / call, full-batch 1KiB
     rows) -> per-group matmuls vs W1 (pairs packed per PSUM bank) ->
     relu+bias -> h1 [8192, nb] bf16 in DRAM (coalesced writes, SP ring)
  C) GL2: dma_gather of perm2 rows -> block-diag W2 pair matmuls ->
     quad-packed h2 tiles resident in SBUF (bf16)
  D) head: 32 accumulating matmuls vs collapsed W3@W4@W5 -> sigmoid -> y

Phases B-D operate on the full per-core batch (fat 1KiB gather rows,
half the instruction count); phase A is split into batch halves and
pipelines against the previous rep's B-D via rotating DRAM staging
buffers (bufs=2). Gathers ride the GPSIMD SWDGE queue (one descriptor
per 16 rows), x loads the SP HWDGE ring, xT writes the ACT ring.
"""

import hashlib

import numpy as np

import concourse.bass as bass
import concourse.mybir as mybir
import concourse.tile as tile
from concourse import bacc, bass_utils, library_config
from concourse.masks import make_identity

G = 128          # groups
C1 = G * 128     # 16384 input channels
C2 = G * 64      # 8192 channels after GL1
N_CORES = 8
B_FULL = 4096

F32 = mybir.dt.float32
BF16 = mybir.dt.bfloat16
I16 = mybir.dt.int16

K1 = 2048        # perm1 idxs per dma_gather call
K2 = 1024        # perm2 idxs per dma_gather call


def build_nc(nb: int, b345: float, reps: int = 1, mid_bf16: int = 1,
             cblk: int = 1024, ablate: int = 0):
    """Build the per-core Bass program for batch-per-core nb.

    reps > 1 repeats the whole pipeline inside one NEFF (for timing:
    device time per rep = (T_reps - T_1) / (reps - 1), cancelling the
    fixed per-execution dispatch overhead).
    """
    nh = 2 if nb >= 512 else 1    # batch halves for phase A
    nbh = nb // nh
    assert nbh % 128 == 0
    nbt = nbh // 128              # batch subtiles of 128 per half
    ncb = C1 // cblk              # channel blocks in phase A
    ncc = cblk // 128             # 128-channel chunks per block

    nc = bacc.Bacc("TRN2", debug=False)
    xs = nc.dram_tensor("xs", [nb, C1], F32, kind="ExternalInput")
    w1 = nc.dram_tensor("w1", [128, G * 64], BF16, kind="ExternalInput")
    w2 = nc.dram_tensor("w2", [128, 64 * 64], BF16, kind="ExternalInput")
    w345 = nc.dram_tensor("w345", [128, 32], BF16, kind="ExternalInput")
    b1p = nc.dram_tensor("b1p", [128, 64], F32, kind="ExternalInput")
    b2q = nc.dram_tensor("b2q", [128, 32], F32, kind="ExternalInput")
    p1g = nc.dram_tensor("p1g", [128, C1 // 16], I16, kind="ExternalInput")
    p2g = nc.dram_tensor("p2g", [128, C2 // 16], I16, kind="ExternalInput")
    y = nc.dram_tensor("y", [1, nb], F32, kind="ExternalOutput")

    relu_t = mybir.ActivationFunctionType.Relu
    sigm_t = mybir.ActivationFunctionType.Sigmoid

    with tile.TileContext(nc) as tc:
        with (
            tc.tile_pool(name="const", bufs=1) as cpool,
            tc.tile_pool(name="h2p", bufs=1) as h2pool,
            tc.tile_pool(name="work", bufs=2) as pool,
            tc.tile_pool(name="psum", bufs=2, space="PSUM") as psum,
            tc.tile_pool(name="dram", bufs=1, space="DRAM") as dpool,
        ):
            # ---- constants / weights preload ----
            w1s = cpool.tile([128, G * 64], BF16)
            nc.sync.dma_start(w1s[:], w1.ap())
            w2s = cpool.tile([128, 64 * 64], BF16)
            nc.sync.dma_start(w2s[:], w2.ap())
            w345s = cpool.tile([128, 32], BF16)
            nc.sync.dma_start(w345s[:], w345.ap())
            b1s = cpool.tile([128, 64], F32)
            nc.sync.dma_start(b1s[:], b1p.ap())
            b2s = cpool.tile([128, 32], F32)
            nc.sync.dma_start(b2s[:], b2q.ap())
            p1s = cpool.tile([128, C1 // 16], I16)
            nc.sync.dma_start(p1s[:], p1g.ap())
            p2s = cpool.tile([128, C2 // 16], I16)
            nc.sync.dma_start(p2s[:], p2g.ap())
            ident = cpool.tile([128, 128], F32)
            make_identity(nc, ident[:])
            ident16 = cpool.tile([128, 128], BF16)
            make_identity(nc, ident16[:])

            nc.gpsimd.load_library(library_config.mlp)

            for _rep in range(reps):
                run_rep(nc, tc, pool, psum, h2pool, dpool, xs, y,
                        w1s, w2s, w345s, b1s, b2s, p1s, p2s, ident, ident16,
                        nb, nh, nbh, nbt, ncb, ncc, cblk, b345,
                        relu_t, sigm_t, ablate)

    nc.compile()
    return nc


def run_rep(nc, tc, pool, psum, h2pool, dpool, xs, y,
            w1s, w2s, w345s, b1s, b2s, p1s, p2s, ident, ident16,
            nb, nh, nbh, nbt, ncb, ncc, cblk, b345, relu_t, sigm_t,
            ablate=0):
    xTd = dpool.tile([C1, nb], BF16, tag="xTd", name="xTd", bufs=2)
    # h1 stays fully SBUF-resident: one wide tile, channel ch at partition
    # ch%128, cols [(ch//128)*nb, (ch//128+1)*nb) -- the rank/token layout
    # the SBUF-source dma_gather addresses directly.
    h1sb = pool.tile([128, (C2 // 128) * nb], BF16, tag="h1sb", bufs=1,
                     name="h1sb")

    # ---- phase A: transpose x -> xT (DRAM, bf16), in batch halves ----
    # 4 transposed 128-channel chunks are packed side-by-side into one wide
    # SBUF tile and written with a single 3D-AP DMA (ACT ring), cutting the
    # HWDGE instruction count 4x vs per-chunk writes.
    for h in range(0 if ablate == 2 else nh):
        b0 = h * nbh
        for cb in range(ncb):
            xn = []
            for bt in range(nbt):
                t = pool.tile([128, cblk], F32, tag=f"xn{bt}", name=f"xn{bt}")
                nc.sync.dma_start(
                    t[:], xs.ap()[b0 + bt * 128:b0 + (bt + 1) * 128,
                                  cb * cblk:(cb + 1) * cblk])
                xn.append(t)
            for cw in range(ncc // 4):
                wide = pool.tile([128, 4 * nbh], BF16, tag="xtw", bufs=3,
                                 name="xtw")
                for cp in range(2):
                    ps_a = psum.tile([128, 2 * nbh], F32, tag="ps_a")
                    for ci in range(2):
                        cc = cw * 4 + cp * 2 + ci
                        for bt in range(nbt):
                            nc.tensor.transpose(
                                ps_a[:, ci * nbh + bt * 128:
                                     ci * nbh + (bt + 1) * 128],
                                xn[bt][:, cc * 128:(cc + 1) * 128],
                                ident[:])
                    nc.vector.tensor_copy(
                        wide[:, cp * 2 * nbh:(cp + 1) * 2 * nbh], ps_a[:])
                ch0 = cb * cblk + cw * 512
                nc.scalar.dma_start(
                    xTd[ch0:ch0 + 512, b0:b0 + nbh].rearrange(
                        "(c p) b -> p c b", c=4), wide[:])

    if ablate == 1:
        yt0 = pool.tile([1, nb], F32, tag="yt", bufs=2, name="yt0")
        nc.vector.memset(yt0[:], 0.0)
        nc.sync.dma_start(y.ap(), yt0[:])
        return

    # ---- phase B: GL1 (batched gather + paired matmuls), full batch ----
    for k in range(C1 // K1):
        g1 = pool.tile([128, K1 // 128, nb], BF16, tag="g1", name="g1")
        nc.gpsimd.dma_gather(
            g1[:], xTd[:], p1s[:, k * (K1 // 16):(k + 1) * (K1 // 16)],
            K1, K1, nb, single_packet=False)
        for j in range(K1 // 256):          # pairs of groups
            pair = (K1 // 256) * k + j
            ps_b = psum.tile([128, nb], F32, tag="ps_b")
            for half in range(2):
                blk = 2 * j + half
                g = (K1 // 128) * k + blk
                nc.tensor.matmul(
                    ps_b[64 * half:64 * (half + 1), :],
                    lhsT=w1s[:, g * 64:(g + 1) * 64], rhs=g1[:, blk, :],
                    start=True, stop=True,
                    tile_position=(0, 64 * half) if half else None)
            nc.scalar.activation(h1sb[:, pair * nb:(pair + 1) * nb],
                                 ps_b[:], relu_t,
                                 bias=b1s[:, pair:pair + 1])

    # ---- phase C: GL2, full batch. perm2 rows come straight out of SBUF
    # via the transpose-mode gather (batch-major output), then PE flips
    # each [128b x 128ch] block back to channel-major for the matmuls.
    h2 = []
    nbc = nb // 128                         # batch 128-blocks
    for k in range(C2 // K2):
        g2t = pool.tile([128, nbc, K2], BF16, tag="g2t", bufs=2, name="g2t")
        nc.gpsimd.dma_gather(
            g2t[:], h1sb[:], p2s[:, k * (K2 // 16):(k + 1) * (K2 // 16)],
            K2, K2, nb, transpose=True, single_packet=False,
            sbuf_tokens_per_rank=128,
            sbuf_free_dim_per_rank=nb * 2,
            sbuf_free_dim_pad_per_rank=0,
            sbuf_byte_offset=0)
        for j in range(K2 // 256):          # quads of groups
            t = (K2 // 256) * k + j
            rhs2 = []
            for half in range(2):
                blk = 2 * j + half
                ps_r = psum.tile([128, nb], BF16, tag="ps_r", bufs=1)
                for c in range(nbc):
                    nc.tensor.transpose(
                        ps_r[:, c * 128:(c + 1) * 128],
                        g2t[:, c, blk * 128:(blk + 1) * 128],
                        ident16[:])
                r2 = pool.tile([128, nb], BF16, tag="rhs2", bufs=4,
                               name="r2")
                nc.vector.tensor_copy(r2[:], ps_r[:])
                rhs2.append(r2)
            ps_c = psum.tile([128, nb], F32, tag="ps_c", bufs=2)
            for half in range(2):
                q = (K2 // 128) * k + 2 * j + half
                nc.tensor.matmul(
                    ps_c[64 * half:64 * (half + 1), :],
                    lhsT=w2s[:, q * 64:(q + 1) * 64], rhs=rhs2[half][:],
                    start=True, stop=True,
                    tile_position=(0, 64 * half) if half else None)
            h2t = h2pool.tile([128, nb], BF16, tag=f"h2_{t}", name=f"h2_{t}")
            nc.scalar.activation(h2t[:], ps_c[:], relu_t,
                                 bias=b2s[:, t:t + 1])
            h2.append(h2t)

    # ---- phase D: head ----
    ps_d = psum.tile([1, nb], F32, tag="ps_d", bufs=1)
    for t in range(32):
        nc.tensor.matmul(ps_d[:], lhsT=w345s[:, t:t + 1], rhs=h2[t][:],
                         start=(t == 0), stop=(t == 31))
    yt = pool.tile([1, nb], F32, tag="yt", bufs=2)
    nc.scalar.activation(yt[:], ps_d[:], sigm_t, bias=float(b345))
    nc.sync.dma_start(y.ap(), yt[:])


def _gather_table(perm: np.ndarray, chunk: int) -> np.ndarray:
    """Index table for dma_gather: chunk c occupies cols [c*chunk/16,
    (c+1)*chunk/16); within a chunk, idx position i = col*16 + row.
    Rows 0-15 hold the indices; replicated to all 128 partitions."""
    n = perm.shape[0]
    cols = chunk // 16
    t = np.zeros((128, (n // chunk) * cols), dtype=np.int16)
    for c in range(n // chunk):
        blk = perm[c * chunk:(c + 1) * chunk].reshape(cols, 16).T  # [16, cols]
        t[:, c * cols:(c + 1) * cols] = np.tile(blk, (8, 1))
    return t


def prep_host(perm1, perm2, W1, b1, W2, b2, W3, b3, W4, b4, W5, b5,
              mid_bf16=1):
    """Host-side layout prep of weights / index tables (replicated per core)."""
    import ml_dtypes
    wdt = ml_dtypes.bfloat16
    w1h = np.ascontiguousarray(
        W1.astype(np.float32).transpose(1, 0, 2).reshape(128, G * 64)).astype(wdt)
    w2h = np.zeros((128, 64 * 64), dtype=wdt)
    for q in range(64):
        w2h[0:64, q * 64:q * 64 + 32] = W2[2 * q].astype(wdt)
        w2h[64:128, q * 64 + 32:(q + 1) * 64] = W2[2 * q + 1].astype(wdt)
    wv = (W3.astype(np.float64) @ W4.astype(np.float64) @ W5.astype(np.float64))
    w345h = np.ascontiguousarray(
        wv.astype(np.float32).reshape(32, 128).T).astype(wdt)
    b345 = float(
        (b3.astype(np.float64) @ W4.astype(np.float64) @ W5.astype(np.float64)
         + b4.astype(np.float64) @ W5.astype(np.float64)
         + b5.astype(np.float64)).reshape(()))
    b1h = np.ascontiguousarray(
        b1.astype(np.float32).reshape(64, 128).T)   # col k = [b1[2k]; b1[2k+1]]
    b2h = np.ascontiguousarray(
        b2.astype(np.float32).reshape(32, 128).T)   # col j = b2[4j:4j+4] stacked
    p1h = _gather_table(perm1.astype(np.int64), K1)
    p2h = _gather_table(perm2.astype(np.int64), K2)
    return {"w1": w1h, "w2": w2h, "w345": w345h, "b1p": b1h, "b2q": b2h,
            "p1g": p1h, "p2g": p2h}, b345


_NC_CACHE: dict = {}

MID_BF16 = 1


def get_nc(nb: int, b345: float, key_bytes: bytes, mid_bf16: int = MID_BF16):
    key = (nb, mid_bf16,
           hashlib.sha256(key_bytes + np.float64(b345).tobytes()).hexdigest())
    if key not in _NC_CACHE:
        _NC_CACHE[key] = build_nc(nb, b345, mid_bf16=mid_bf16)
    return _NC_CACHE[key]


def kernel(x, perm1, perm2, W1, b1, W2, b2, W3, b3, W4, b4, W5, b5):
    x = np.asarray(x)
    consts, b345 = prep_host(np.asarray(perm1), np.asarray(perm2),
                             np.asarray(W1), np.asarray(b1), np.asarray(W2),
                             np.asarray(b2), np.asarray(W3), np.asarray(b3),
                             np.asarray(W4), np.asarray(b4), np.asarray(W5),
                             np.asarray(b5), mid_bf16=MID_BF16)
    nb = x.shape[0] // N_CORES
    key_bytes = np.asarray(perm1).tobytes() + np.asarray(perm2).tobytes()
    nc = get_nc(nb, b345, key_bytes)
    in_maps = []
    for c in range(N_CORES):
        m = dict(consts)
        m["xs"] = np.ascontiguousarray(x[c * nb:(c + 1) * nb].astype(np.float32))
        in_maps.append(m)
    res = bass_utils.run_bass_kernel_spmd(nc, in_maps, core_ids=list(range(N_CORES)))
    out = np.concatenate([res.results[c]["y"].reshape(nb) for c in range(N_CORES)])
    return out.reshape(-1, 1).astype(np.float32)
